# revision 6
# baseline (speedup 1.0000x reference)
"""MultiHeadAttention Trainium2 kernel (v3: gap-free exp stream pipeline).

Full inputs: x [4, 2048, 768] f32, W_qkv [2304, 768], W_proj [768, 768],
b_proj [768]. Output [4, 2048, 768] f32.

Sharding: 8 cores = 4 batches x 2 head-groups (6 heads each).
Per-core inputs (host-prepared, transposed on host):
  xT  [768, 2048]  = x[b].T
  wT  [768, 1152]  = concat(Wq_g, Wk_g, Wv_g).T   (g = head group rows)
  wpT [384, 768]   = W_proj[:, g-cols].T
  identT [128,128] = identity (for PE transposes)
Per-core output: outp [2048, 768] = partial projection output for batch b.
Host: out[b] = outp[2b] + outp[2b+1] + b_proj.

Design: the scalar-engine exp stream (~201us: 21 instrs/head over a 6-bank
PSUM ring, groups of 3x512-f32 banks -> one [128,1536] activation) is the
binding resource; the kernel keeps it gap-free from ~16us on. All other PE
work (QKV projection chains, flipped-AV chains, att transposes, output
projection) is emitted as split "filler units" interleaved between QK
matmuls under a per-head credit pacing model, so the in-order PE both feeds
the ring on time and never idles long enough to trip the HAM clock gate
(idle > ~3.4us halves the PE utilization cap).

AV is flipped: stationary = e chunk [128k x 128q] (LDWEIGHTS pipelines
behind the 65-col moving v_aug whose ones-column yields the softmax
denominator in av[:, 64]). av [q, 65] makes the reciprocal a per-partition
scalar (reciprocal_approx_fast) and the normalize one tensor_scalar_mul.
att [q, hd] is PE-transposed per head-pair into attT [hd, q] for the
output projection.
"""

from collections import deque

import ml_dtypes
import numpy as np

import concourse.bass as bass
import concourse.tile as tile
from concourse import bacc, mybir
from concourse.bass_utils import run_bass_kernel_spmd

EMB = 768
N = 2048
B = 4
D = 64
HL = 6            # heads per core
HD = HL * D       # 384 local head-dim columns
NCORES = 8
SCALE = D ** -0.5

F32 = mybir.dt.float32
BF16 = mybir.dt.bfloat16

EC = EMB // 128   # 6 emb chunks
MC = HD // 128    # 3 local head-dim chunks
NQ = N // 512     # 4 query col-blocks of 512
NK = N // 128     # 16 key/seq chunks of 128
ER = 80           # e_ring slots (sbuf)
RS = 5            # psum ring slots

EXP = mybir.ActivationFunctionType.Exp

# PE-time estimates (ns) for pacing filler emission between QK matmuls.
C_QKHALF = 650    # 3 x 512-col matmuls (half a qkv-projection chain)
C_VHALF = 490     # 3 x 384-col matmuls (half a v chain)
C_AVHALF = 330    # 8 x 65-col matmuls (half a flipped-AV chain)
C_TPPAIR = 110    # 2 PE transposes
# Per-head filler credit granted per QK matmul emitted (ns).
RATE = [550, 380, 330, 330, 330, 330]
C_JUNK = 213      # one 512-col junk matmul (PE padding vs HAM downshift)


def _emit(tc):
    from contextlib import ExitStack

    nc = tc.nc
    xT = nc.dram_tensor("xT", [EMB, N], BF16, kind="ExternalInput").ap()
    wT = nc.dram_tensor("wT", [EMB, 3 * HD], BF16, kind="ExternalInput").ap()
    wpT = nc.dram_tensor("wpT", [HD, EMB], BF16, kind="ExternalInput").ap()
    identT = nc.dram_tensor("identT", [128, 128], BF16, kind="ExternalInput").ap()
    outp = nc.dram_tensor("outp", [N, EMB], F32, kind="ExternalOutput").ap()

    xTr = xT.rearrange("(c p) s -> p c s", p=128)
    wTr = wT.rearrange("(c p) s -> p c s", p=128)
    wpTr = wpT.rearrange("(m p) e -> p m e", p=128)
    outr = outp.rearrange("(s p) e -> p s e", p=128)

    with ExitStack() as persist:
        pp = persist.enter_context(tc.tile_pool(name="persist", bufs=1))
        warm_sb = pp.tile([128, 640], BF16)
        nc.vector.memset(warm_sb[:], 1.0)
        wp_sb = pp.tile([128, MC, EMB], BF16)
        ident_sb = pp.tile([128, 128], BF16)
        qT_sb = pp.tile([128, MC, N], BF16)
        kT_sb = pp.tile([128, HL, N], BF16)
        nc.vector.memset(kT_sb[:], 0.0)
        v_sb = pp.tile([128, NK, HL * (D + 1)], BF16)
        nc.vector.memset(v_sb[:], 1.0)
        attT_sb = pp.tile([128, MC, N], BF16)
        att_pair = pp.tile([128, 2, NK, 128], BF16)
        rec_sb = pp.tile([128, 2, NK, 1], F32)
        e_ring = pp.tile([128, ER, 512], BF16)

        psum = persist.enter_context(
            tc.tile_pool(name="psum", bufs=1, space="PSUM"))
        ring_ps = psum.tile([128, RS, 512], F32, name="ring_ps")
        junk_ps = psum.tile([128, 512], F32, tag="junk", bufs=1,
                            name="junk_ps")

        def junk_mm():
            nc.tensor.matmul(junk_ps[:], warm_sb[:, 0:128],
                             warm_sb[:, 128:640], start=True, stop=True)

        def sm_tile(name, dtype=F32, shape=(128, 512)):
            return psum.tile(list(shape), dtype, tag="sm", bufs=2, name=name)

        ph1 = persist.enter_context(tc.tile_pool(name="ph1", bufs=1))
        x_sb = ph1.tile([128, EC, N], BF16)
        w_sb = ph1.tile([128, EC, 3 * HD], BF16)
        # Split input DMAs across two engine queues (x on sync, w on gpsimd).
        nc.gpsimd.dma_start(wp_sb[:], wpTr)
        nc.gpsimd.dma_start(ident_sb[:], identT)
        for c in range(EC):
            nc.gpsimd.dma_start(w_sb[:, c, :], wTr[:, c, :])
            nc.sync.dma_start(x_sb[:, c, :], xTr[:, c, :])

        # PE warmup: cover the input-DMA window so the HAM clock gate is
        # open and the p-state ramped when real work starts.
        for wi in range(24):
            junk_mm()

        # ---- emitters ----
        qk_state = {}

        def qk_half(which, m, qn, half):
            # half 0/1 of a 6-matmul qkv projection chain (contract over EC)
            key = (which, m, qn)
            sl = slice(qn * 512, (qn + 1) * 512)
            if half == 0:
                qk_state[key] = sm_tile(f"qk_{which}_{m}_{qn}")
            t = qk_state[key]
            lo = which * HD + m * 128
            for c in range(3 * half, 3 * half + 3):
                nc.tensor.matmul(t[:], w_sb[:, c, lo:lo + 128],
                                 x_sb[:, c, sl],
                                 start=(c == 0), stop=(c == EC - 1))
            if half == 1:
                if which == 0:
                    nc.vector.tensor_copy(qT_sb[:, m, sl], t[:])
                else:
                    nc.vector.tensor_copy(kT_sb[0:64, 2 * m, sl], t[0:64, :])
                    nc.vector.tensor_copy(kT_sb[64:128, 2 * m + 1, sl],
                                          t[64:128, :])
                del qk_state[key]

        v_state = {}

        def v_half(s, half):
            if half == 0:
                v_state[s] = sm_tile(f"v_{s}")
            tv = v_state[s][:, 0:HD]
            for c in range(3 * half, 3 * half + 3):
                nc.tensor.matmul(tv, x_sb[:, c, s * 128:(s + 1) * 128],
                                 w_sb[:, c, 2 * HD:3 * HD],
                                 start=(c == 0), stop=(c == EC - 1))
            if half == 1:
                nc.vector.tensor_copy(
                    v_sb[:, s, :].rearrange("p (h c) -> p h c",
                                            c=D + 1)[:, :, 0:D],
                    tv.rearrange("p (h d) -> p h d", h=HL))
                del v_state[s]

        av_state = {}

        def av_half(h, qc, half):
            m, p0 = h // 2, (h % 2) * 64
            qn, j = divmod(qc, 4)
            e0 = (64 * h) % ER
            if half == 0:
                av_state[(h, qc)] = sm_tile(f"av_{h}_{qc}")
            av = av_state[(h, qc)]
            for kk in range(8 * half, 8 * half + 8):
                eidx = (e0 + qn * 16 + kk) % ER
                nc.tensor.matmul(av[:, 0:D + 1],
                                 e_ring[:, eidx, j * 128:(j + 1) * 128],
                                 v_sb[:, kk, h * (D + 1):(h + 1) * (D + 1)],
                                 start=(kk == 0), stop=(kk == NK - 1))
            if half == 1:
                nc.vector.reciprocal_approx_fast(out=rec_sb[:, h % 2, qc, :],
                                                 in_=av[:, D:D + 1])
                nc.vector.tensor_scalar_mul(att_pair[:, m % 2, qc, p0:p0 + D],
                                            av[:, 0:D],
                                            rec_sb[:, h % 2, qc, :])
                del av_state[(h, qc)]

        def tp_pair(m, qc):
            for q in (qc, qc + 1):
                t = sm_tile(f"tp_{m}_{q}", dtype=BF16, shape=(128, 128))
                nc.tensor.transpose(t[:], att_pair[:, m % 2, q, :],
                                    ident_sb[:])
                nc.vector.tensor_copy(attT_sb[:, m, q * 128:(q + 1) * 128],
                                      t[:])

        def ph3(s):
            o_sb = ph1.tile([128, EMB], F32, tag="osb", bufs=3,
                            name=f"osb_{s}")
            for half in range(2):
                pr = sm_tile(f"pr_{s}_{half}")
                prr = pr[:, 0:HD]
                for mi in range(MC):
                    nc.tensor.matmul(prr,
                                     attT_sb[:, mi, s * 128:(s + 1) * 128],
                                     wp_sb[:, mi, half * HD:(half + 1) * HD],
                                     start=(mi == 0), stop=(mi == MC - 1))
                nc.vector.tensor_copy(o_sb[:, half * HD:(half + 1) * HD], prr)
            nc.sync.dma_start(outr[:, s, :], o_sb[:])

        # ---- filler queue + pacing ----
        fillq = deque()

        def push_qk(which, m, qn):
            fillq.append((C_QKHALF, lambda: qk_half(which, m, qn, 0)))
            fillq.append((C_QKHALF, lambda: qk_half(which, m, qn, 1)))

        def push_v(s):
            fillq.append((C_VHALF, lambda: v_half(s, 0)))
            fillq.append((C_VHALF, lambda: v_half(s, 1)))

        def push_chain(h, qc):
            fillq.append((C_AVHALF, lambda: av_half(h, qc, 0)))
            fillq.append((C_AVHALF, lambda: av_half(h, qc, 1)))

        def push_tp(m, qc):
            fillq.append((C_TPPAIR, lambda: tp_pair(m, qc)))

        # prelude: kT(m0) qn0 + qT(m0) qn0 fully, before head 0's QK loop
        for half in range(2):
            qk_half(1, 0, 0, half)
        for half in range(2):
            qk_half(0, 0, 0, half)

        groups = []
        for b5 in range(12):
            groups.append((5 * b5, 5 * b5 + 2))
            groups.append((5 * b5 + 3, 5 * b5 + 4))
        groups.append((60, 63))

        def emit_exp(h, g0, g1):
            e0 = (64 * h) % ER
            s6 = g0 % RS
            n = g1 - g0 + 1
            d0 = (e0 + g0) % ER
            if d0 + n <= ER:
                nc.scalar.activation(e_ring[:, d0:d0 + n, :],
                                     ring_ps[:, s6:s6 + n, :], EXP,
                                     scale=SCALE)
            else:
                k = ER - d0
                nc.scalar.activation(e_ring[:, d0:d0 + k, :],
                                     ring_ps[:, s6:s6 + k, :], EXP,
                                     scale=SCALE)
                nc.scalar.activation(e_ring[:, 0:n - k, :],
                                     ring_ps[:, s6 + k:s6 + n, :], EXP,
                                     scale=SCALE)

        for h in range(HL):
            m = h // 2
            # push this head's filler units (deadline order)
            if h == 0:
                for qn in (1, 2, 3):
                    push_qk(1, 0, qn)      # kT(m0) rest: needed at kk=4qn
                for qn in (1, 2, 3):
                    push_qk(0, 0, qn)      # qT(m0) rest: needed at quarter qn
                for s in range(NK):
                    push_v(s)
            elif h == 1:
                for qc in range(NK):
                    push_chain(0, qc)
                for qn in range(NQ):
                    push_qk(1, 1, qn)
                for qn in range(NQ):
                    push_qk(0, 1, qn)
            elif h == 2:
                for qn2 in range(2):
                    for qc in range(8 * qn2, 8 * qn2 + 8):
                        push_chain(1, qc)
                    for qn in range(2 * qn2, 2 * qn2 + 2):
                        push_qk(1, 2, qn)
                        push_qk(0, 2, qn)
            elif h == 3:
                for qc in range(NK):
                    push_chain(2, qc)
                    if qc % 2 == 1:
                        push_tp(0, qc - 1)
            elif h == 4:
                for qc in range(NK):
                    push_chain(3, qc)
            elif h == 5:
                for qc in range(NK):
                    push_chain(4, qc)
                    if qc % 2 == 1:
                        push_tp(1, qc - 1)

            gi = 0
            credit = 0
            for qn in range(NQ):
                if h == 5 and qn >= 1:
                    for qc in range(4 * (qn - 1), 4 * qn):
                        push_chain(5, qc)
                for kk in range(NK):
                    L = qn * 16 + kk
                    slot = L % RS
                    nc.tensor.matmul(ring_ps[:, slot, :],
                                     kT_sb[:, h, kk * 128:(kk + 1) * 128],
                                     qT_sb[:, m, qn * 512:(qn + 1) * 512],
                                     start=True, stop=True)
                    while gi < len(groups) and groups[gi][1] == L:
                        emit_exp(h, *groups[gi])
                        gi += 1
                    credit += RATE[h]
                    while fillq and credit >= fillq[0][0]:
                        cost, fn = fillq.popleft()
                        fn()
                        credit -= cost
                    if not fillq:
                        # keep the PE array busy so the HAM activity monitor
                        # never halves the utilization cap
                        while credit >= C_JUNK:
                            junk_mm()
                            credit -= C_JUNK

        # ---- tail ----
        for qc in range(12, 16):
            push_chain(5, qc)
        while fillq:
            _, fn = fillq.popleft()
            fn()
        for qc in range(0, NK, 2):
            tp_pair(2, qc)
            ph3(qc)
            ph3(qc + 1)


_CACHE = {}


def _build():
    if "nc" not in _CACHE:
        nc = bacc.Bacc("TRN2", target_bir_lowering=False, debug=False,
                       num_devices=NCORES)
        with tile.TileContext(nc) as tc:
            _emit(tc)
        nc.compile()
        _CACHE["nc"] = nc
    return _CACHE["nc"]


def _in_maps(x, W_qkv, W_proj):
    bf = ml_dtypes.bfloat16
    ident = np.eye(128, dtype=bf)
    in_maps = []
    for c in range(NCORES):
        b, g = divmod(c, 2)
        r0 = g * HD
        w_rows = np.concatenate([
            W_qkv[0 * EMB + r0: 0 * EMB + r0 + HD],
            W_qkv[1 * EMB + r0: 1 * EMB + r0 + HD],
            W_qkv[2 * EMB + r0: 2 * EMB + r0 + HD],
        ], axis=0)                                   # [1152, 768]
        in_maps.append({
            "xT": np.ascontiguousarray(x[b].T.astype(bf)),
            "wT": np.ascontiguousarray(w_rows.T.astype(bf)),
            "wpT": np.ascontiguousarray(W_proj[:, r0:r0 + HD].T.astype(bf)),
            "identT": ident,
        })
    return in_maps


LAST_RESULTS = None


def kernel(x, W_qkv, W_proj, b_proj):
    global LAST_RESULTS
    x = np.ascontiguousarray(np.asarray(x, dtype=np.float32))
    W_qkv = np.asarray(W_qkv, dtype=np.float32)
    W_proj = np.asarray(W_proj, dtype=np.float32)
    b_proj = np.asarray(b_proj, dtype=np.float32)

    nc = _build()
    in_maps = _in_maps(x, W_qkv, W_proj)
    res = run_bass_kernel_spmd(nc, in_maps, core_ids=list(range(NCORES)))
    LAST_RESULTS = res

    out = np.empty((B, N, EMB), dtype=np.float32)
    for b in range(B):
        out[b] = res.results[2 * b]["outp"] + res.results[2 * b + 1]["outp"]
    out += b_proj
    return out


# revision 9
# speedup vs baseline: 1.4828x; 1.4828x over previous
"""MultiHeadAttention Trainium2 kernel.

Full inputs: x [4, 2048, 768] f32, W_qkv [2304, 768], W_proj [768, 768],
b_proj [768]. Output [4, 2048, 768] f32.

Sharding: 8 cores = 4 batches x 2 head-groups (6 heads each).
Per-core inputs (host-prepared, transposed on host):
  xT  [768, 2048]  = x[b].T
  wT  [768, 1152]  = concat(Wq_g, Wk_g, Wv_g).T   (g = head group rows)
  wpT [384, 768]   = W_proj[:, g-cols].T
Per-core output: outp [2048, 768] = partial projection output for batch b.
Host: out[b] = outp[2b] + outp[2b+1] + b_proj.

On-device (per core):
  phase 1: qT/kT [384, 2048] (head-dim on partitions) and v [2048, 384+ones]
           via f32r matmuls; x.T and W.T arrive pre-transposed from host.
  phase 2: per (head, k-chunk): energyT[k,q] = kT.T @ qT (K=64), one exp
           activation over 4 psum banks (scale=1/8 folded in, no max
           subtraction -- energies are O(+-10) for this distribution), then
           av[65, q] += v_aug.T @ e accumulated over k-chunks. Row 64 of av
           is the softmax denominator (ones column of v_aug).
           attT[hd, q] = av[0:64] * (1/l broadcast).
  phase 3: out[s, e] = attT.T @ wpT accumulated over hd-chunks -> DMA out.
"""

import ml_dtypes
import numpy as np

import concourse.bass as bass
import concourse.tile as tile
from concourse import bacc, mybir
from concourse.bass_utils import run_bass_kernel_spmd

EMB = 768
N = 2048
B = 4
D = 64
HL = 6            # heads per core
HD = HL * D       # 384 local head-dim columns
NCORES = 8
SCALE = D ** -0.5

F32 = mybir.dt.float32
BF16 = mybir.dt.bfloat16

EC = EMB // 128   # 6 emb chunks
MC = HD // 128    # 3 local head-dim chunks
NQ = N // 512     # 4 query chunks of 512
NK = N // 128     # 16 key/seq chunks of 128

EXP = mybir.ActivationFunctionType.Exp


def _emit(tc):
    from contextlib import ExitStack

    nc = tc.nc
    xT = nc.dram_tensor("xT", [EMB, N], BF16, kind="ExternalInput").ap()
    wT = nc.dram_tensor("wT", [EMB, 3 * HD], BF16, kind="ExternalInput").ap()
    wpT = nc.dram_tensor("wpT", [HD, EMB], BF16, kind="ExternalInput").ap()
    outp = nc.dram_tensor("outp", [N, EMB], F32, kind="ExternalOutput").ap()

    xTr = xT.rearrange("(c p) s -> p c s", p=128)
    wTr = wT.rearrange("(c p) s -> p c s", p=128)
    wpTr = wpT.rearrange("(m p) e -> p m e", p=128)
    outr = outp.rearrange("(s p) e -> p s e", p=128)

    with ExitStack() as persist:
        ppool = persist.enter_context(tc.tile_pool(name="persist", bufs=1))
        psum_pool0 = None  # placeholder, real pool created below
        # PE warmup: ~4us of junk matmuls, emitted first so they run during
        # the input-DMA wait and open the HAM clock-gate before real work
        warm_sb = ppool.tile([128, 640], BF16)
        nc.vector.memset(warm_sb[:], 1.0)
        wp_sb = ppool.tile([128, MC, EMB], BF16)
        nc.sync.dma_start(wp_sb[:], wpTr)
        qT_sb = ppool.tile([128, MC, N], BF16)
        kT_sb = ppool.tile([128, HL, N], BF16)
        nc.vector.memset(kT_sb[:], 0.0)
        v_sb = ppool.tile([128, NK, HL * (D + 1) + D], BF16)
        nc.vector.memset(v_sb[:], 1.0)
        attT_sb = ppool.tile([128, MC, N], BF16)

        psum_pool = persist.enter_context(
            tc.tile_pool(name="psum", bufs=1, space="PSUM"))
        warm_ps = psum_pool.tile([128, 512], F32, tag="av", bufs=4, name="warm_ps")
        for wi in range(10):
            nc.tensor.matmul(warm_ps[:], warm_sb[:, 0:128], warm_sb[:, 128:640],
                             start=(wi == 0), stop=(wi == 9))

        # ---- phase 1: qkv projection (m0 + v0/v1 up front; rest streams
        # into the phase-2 slack so the exp stream starts ~40us earlier) ----
        p1 = persist.enter_context(tc.tile_pool(name="ph1", bufs=1))
        x_sb = p1.tile([128, EC, N], BF16)
        w_sb = p1.tile([128, EC, 3 * HD], BF16)
        for c in range(EC):
            nc.sync.dma_start(w_sb[:, c, :], wTr[:, c, :])
            nc.sync.dma_start(x_sb[:, c, :], xTr[:, c, :])

        def qk_chain(which, m, n, tag="av", bufs=4):
            lo = which * HD + m * 128
            if tag == "eps":
                mm = psum_pool.tile([128, 2, 512], F32, tag=tag, bufs=bufs,
                                    name=f"mm_{which}_{m}_{n}")[:, 0, :]
            else:
                mm = psum_pool.tile([128, 512], F32, tag=tag, bufs=bufs,
                                    name=f"mm_{which}_{m}_{n}")
            for c in range(EC):
                nc.tensor.matmul(
                    mm[:],
                    (w_sb[:, c, lo:lo + 128]),
                    (x_sb[:, c, n * 512:(n + 1) * 512]),
                    start=(c == 0), stop=(c == EC - 1))
            ns = slice(n * 512, (n + 1) * 512)
            if which == 0:
                nc.vector.tensor_copy(qT_sb[:, m, ns], mm[:])
            else:
                nc.vector.tensor_copy(kT_sb[0:64, 2 * m, ns], mm[0:64, :])
                nc.vector.tensor_copy(kT_sb[64:128, 2 * m + 1, ns], mm[64:128, :])

        def v_chain(s):
            vv = psum_pool.tile([128, 2, 512], F32, tag="eps", bufs=2,
                                name=f"vv_{s}")[:, 0, 0:HD]
            for c in range(EC):
                nc.tensor.matmul(
                    vv[:],
                    (x_sb[:, c, s * 128:(s + 1) * 128]),
                    (w_sb[:, c, 2 * HD:3 * HD]),
                    start=(c == 0), stop=(c == EC - 1))
            nc.vector.tensor_copy(
                v_sb[:, s, 0:HL * (D + 1)].rearrange(
                    "p (h c) -> p h c", c=D + 1)[:, :, 0:D],
                vv[:].rearrange("p (h d) -> p h d", h=HL))

        for which in (0, 1):
            for n in range(NQ):
                qk_chain(which, 0, n)
        for s in (0, 1):
            v_chain(s)

        # ---- phase 2: attention ----
        with ExitStack() as ph2:
            esb_pool = ph2.enter_context(tc.tile_pool(name="esb", bufs=4))
            sm_pool = ph2.enter_context(tc.tile_pool(name="sm", bufs=4))

            for h in range(HL):
                m, p0 = h // 2, (h % 2) * 64
                avs = [psum_pool.tile([128, 512], F32, tag="av", bufs=4, name=f"av_{h}_{n}")
                       for n in range(NQ)]
                for kk in range(NK):
                    # two 2-bank energy tiles per kk so the next group's QK
                    # overlaps this group's exp (keeps the PE array gap-free;
                    # periodic array idles re-throttle the HAM clock gate)
                    e_sbs = []
                    for half in range(2):
                        e_ps = psum_pool.tile([128, 2, 512], F32, tag="eps", bufs=2,
                                             name=f"eps_{h}_{kk}_{half}")
                        for j in range(2):
                            n = half * 2 + j
                            nc.tensor.matmul(
                                e_ps[:, j, :],
                                (kT_sb[:, h, kk * 128:(kk + 1) * 128]),
                                (qT_sb[0:128, m, n * 512:(n + 1) * 512]),
                                start=True, stop=True)
                        e_sb = esb_pool.tile([128, 2, 512], BF16, tag="esb",
                                             name=f"esb_{h}_{kk}_{half}")
                        nc.scalar.activation(e_sb[:], e_ps[:], EXP, scale=SCALE)
                        e_sbs.append(e_sb)
                    for n in range(NQ):
                        nc.tensor.matmul(
                            avs[n][:],
                            (v_sb[:, kk, h * (D + 1): h * (D + 1) + 128]),
                            (e_sbs[n // 2][:, n % 2, :]),
                            start=(kk == 0), stop=(kk == NK - 1))
                    if h == 0 and kk <= 13:
                        v_chain(kk + 2)
                    elif h in (1, 2) and kk < 8:
                        qk_chain(0 if kk >= 4 else 1, h, kk % 4,
                                 tag="eps", bufs=2)
                # drain all four av banks first (the slow reciprocals would
                # otherwise sit ahead of the copies in the DVE queue and stall
                # the next head's AV accumulation on bank reuse). For the last
                # head, run per-n chains with the drain on the (idle) scalar
                # engine so phase 3 unblocks sooner.
                last = h == HL - 1
                avsts = []
                for n in range(NQ):
                    avst = sm_pool.tile([D + 1, 512], F32, tag="avst", bufs=8,
                                        name=f"avst_{h}_{n}")
                    if last:
                        nc.scalar.copy(avst[:], avs[n][0:D + 1, :])
                    else:
                        nc.vector.tensor_copy(avst[:], avs[n][0:D + 1, :])
                    avsts.append(avst)
                for n in range(NQ):
                    rec = sm_pool.tile([1, 512], F32, tag="rec", bufs=8,
                                       name=f"rec_{h}_{n}")
                    nc.vector.reciprocal(rec[:], avsts[n][D:D + 1, :])
                    rb = sm_pool.tile([D, 512], F32, tag="rb", bufs=8,
                                      name=f"rb_{h}_{n}")
                    nc.gpsimd.partition_broadcast(rb[:], rec[:])
                    nc.vector.tensor_mul(
                        attT_sb[p0:p0 + 64, m, n * 512:(n + 1) * 512],
                        avsts[n][0:D, :], rb[:])

        # keep the PE array busy through the last head's normalization tail
        # (an idle window >3.4us here re-throttles the clock for phase 3)
        fill_ps = psum_pool.tile([128, 512], F32, tag="av", bufs=4, name="fill_ps")
        for wi in range(40):
            nc.tensor.matmul(fill_ps[:], warm_sb[:, 0:128], warm_sb[:, 128:640],
                             start=(wi == 0), stop=(wi == 39))

        # ---- phase 3: output projection (natural layout) ----
        with ExitStack() as ph3:
            osb_pool = ph3.enter_context(tc.tile_pool(name="osb", bufs=3))
            for s in range(NK):
                o_sb = osb_pool.tile([128, EMB], F32, tag="osb", name=f"osb_{s}")
                for half in range(2):
                    pr = psum_pool.tile([128, 512], F32, tag="av", bufs=4, name=f"pr_{s}_{half}")[:, 0:HD]
                    for m in range(MC):
                        nc.tensor.matmul(
                            pr[:],
                            (attT_sb[:, m, s * 128:(s + 1) * 128]),
                            (wp_sb[:, m, half * HD:(half + 1) * HD]),
                            start=(m == 0), stop=(m == MC - 1))
                    nc.vector.tensor_copy(o_sb[:, half * HD:(half + 1) * HD], pr[:])
                nc.sync.dma_start(outr[:, s, :], o_sb[:])


_CACHE = {}


def _build():
    if "nc" not in _CACHE:
        nc = bacc.Bacc("TRN2", target_bir_lowering=False, debug=False,
                       num_devices=NCORES)
        with tile.TileContext(nc) as tc:
            _emit(tc)
        nc.compile()
        _CACHE["nc"] = nc
    return _CACHE["nc"]


def _in_maps(x, W_qkv, W_proj):
    in_maps = []
    for c in range(NCORES):
        b, g = divmod(c, 2)
        r0 = g * HD
        w_rows = np.concatenate([
            W_qkv[0 * EMB + r0: 0 * EMB + r0 + HD],
            W_qkv[1 * EMB + r0: 1 * EMB + r0 + HD],
            W_qkv[2 * EMB + r0: 2 * EMB + r0 + HD],
        ], axis=0)                                   # [1152, 768]
        bf = ml_dtypes.bfloat16
        in_maps.append({
            "xT": np.ascontiguousarray(x[b].T.astype(bf)),
            "wT": np.ascontiguousarray(w_rows.T.astype(bf)),
            "wpT": np.ascontiguousarray(W_proj[:, r0:r0 + HD].T.astype(bf)),
        })
    return in_maps


LAST_RESULTS = None


def kernel(x, W_qkv, W_proj, b_proj):
    global LAST_RESULTS
    x = np.ascontiguousarray(np.asarray(x, dtype=np.float32))
    W_qkv = np.asarray(W_qkv, dtype=np.float32)
    W_proj = np.asarray(W_proj, dtype=np.float32)
    b_proj = np.asarray(b_proj, dtype=np.float32)

    nc = _build()
    in_maps = _in_maps(x, W_qkv, W_proj)
    res = run_bass_kernel_spmd(nc, in_maps, core_ids=list(range(NCORES)))
    LAST_RESULTS = res

    out = np.empty((B, N, EMB), dtype=np.float32)
    for b in range(B):
        out[b] = res.results[2 * b]["outp"] + res.results[2 * b + 1]["outp"]
    out += b_proj
    return out



# revision 10
# speedup vs baseline: 1.6877x; 1.1382x over previous
"""MultiHeadAttention Trainium2 kernel.

Full inputs: x [4, 2048, 768] f32, W_qkv [2304, 768], W_proj [768, 768],
b_proj [768]. Output [4, 2048, 768] f32.

Sharding: 8 cores = 4 batches x 2 head-groups (6 heads each).
Per-core inputs (host-prepared, transposed on host):
  xT  [768, 2048]  = x[b].T
  wT  [768, 1152]  = concat(Wq_g, Wk_g, Wv_g).T   (g = head group rows)
  wpT [384, 768]   = W_proj[:, g-cols].T
Per-core output: outp [2048, 768] = partial projection output for batch b.
Host: out[b] = outp[2b] + outp[2b+1] + b_proj.

On-device (per core):
  phase 1: qT/kT [384, 2048] (head-dim on partitions) and v [2048, 384+ones]
           via f32r matmuls; x.T and W.T arrive pre-transposed from host.
  phase 2: per (head, k-chunk): energyT[k,q] = kT.T @ qT (K=64), one exp
           activation over 4 psum banks (scale=1/8 folded in, no max
           subtraction -- energies are O(+-10) for this distribution), then
           av[65, q] += v_aug.T @ e accumulated over k-chunks. Row 64 of av
           is the softmax denominator (ones column of v_aug).
           attT[hd, q] = av[0:64] * (1/l broadcast).
  phase 3: out[s, e] = attT.T @ wpT accumulated over hd-chunks -> DMA out.
"""

import ml_dtypes
import numpy as np

import concourse.bass as bass
import concourse.tile as tile
from concourse import bacc, mybir
from concourse.bass_utils import run_bass_kernel_spmd

EMB = 768
N = 2048
B = 4
D = 64
HL = 6            # heads per core
HD = HL * D       # 384 local head-dim columns
NCORES = 8
SCALE = D ** -0.5

F32 = mybir.dt.float32
BF16 = mybir.dt.bfloat16

EC = EMB // 128   # 6 emb chunks
MC = HD // 128    # 3 local head-dim chunks
NQ = N // 512     # 4 query chunks of 512
NK = N // 128     # 16 key/seq chunks of 128

EXP = mybir.ActivationFunctionType.Exp


def _emit(tc):
    from contextlib import ExitStack

    nc = tc.nc
    xT = nc.dram_tensor("xT", [EMB, N], BF16, kind="ExternalInput").ap()
    wT = nc.dram_tensor("wT", [EMB, 3 * HD], BF16, kind="ExternalInput").ap()
    wpT = nc.dram_tensor("wpT", [HD, EMB], BF16, kind="ExternalInput").ap()
    outp = nc.dram_tensor("outp", [N, EMB], F32, kind="ExternalOutput").ap()

    xTr = xT.rearrange("(c p) s -> p c s", p=128)
    wTr = wT.rearrange("(c p) s -> p c s", p=128)
    wpTr = wpT.rearrange("(m p) e -> p m e", p=128)
    outr = outp.rearrange("(s p) e -> p s e", p=128)

    with ExitStack() as persist:
        ppool = persist.enter_context(tc.tile_pool(name="persist", bufs=1))
        psum_pool0 = None  # placeholder, real pool created below
        # PE warmup: ~4us of junk matmuls, emitted first so they run during
        # the input-DMA wait and open the HAM clock-gate before real work
        warm_sb = ppool.tile([128, 640], BF16)
        nc.vector.memset(warm_sb[:], 1.0)
        wp_sb = ppool.tile([128, MC, EMB], BF16)
        nc.sync.dma_start(wp_sb[:], wpTr)
        qT_sb = ppool.tile([128, MC, N], BF16)
        kT_sb = ppool.tile([128, HL, N], BF16)
        nc.vector.memset(kT_sb[:], 0.0)
        v_sb = ppool.tile([128, NK, HL * (D + 1) + D], BF16)
        nc.vector.memset(v_sb[:], 1.0)
        attT_sb = ppool.tile([128, MC, N], BF16)

        psum_pool = persist.enter_context(
            tc.tile_pool(name="psum", bufs=1, space="PSUM"))
        warm_ps = psum_pool.tile([128, 512], F32, tag="av", bufs=4, name="warm_ps")
        for wi in range(10):
            nc.tensor.matmul(warm_ps[:], warm_sb[:, 0:128], warm_sb[:, 128:640],
                             start=(wi == 0), stop=(wi == 9))

        # ---- phase 1: qkv projection ----
        with ExitStack() as ph1:
            p1 = ph1.enter_context(tc.tile_pool(name="ph1", bufs=1))
            x_sb = p1.tile([128, EC, N], BF16)
            w_sb = p1.tile([128, EC, 3 * HD], BF16)
            for c in range(EC):
                nc.sync.dma_start(w_sb[:, c, :], wTr[:, c, :])
                nc.sync.dma_start(x_sb[:, c, :], xTr[:, c, :])

            for which in (0, 1):
                for m in range(MC):
                    lo = which * HD + m * 128
                    for n in range(NQ):
                        mm = psum_pool.tile([128, 512], F32, tag="av", bufs=4, name=f"mm_{which}_{m}_{n}")
                        for c in range(EC):
                            nc.tensor.matmul(
                                mm[:],
                                (w_sb[:, c, lo:lo + 128]),
                                (x_sb[:, c, n * 512:(n + 1) * 512]),
                                start=(c == 0), stop=(c == EC - 1))
                        ns = slice(n * 512, (n + 1) * 512)
                        if which == 0:
                            nc.vector.tensor_copy(qT_sb[:, m, ns], mm[:])
                        else:
                            nc.vector.tensor_copy(kT_sb[0:64, 2 * m, ns], mm[0:64, :])
                            nc.vector.tensor_copy(kT_sb[64:128, 2 * m + 1, ns], mm[64:128, :])

            for s in range(NK):
                vv = psum_pool.tile([128, 2, 512], F32, tag="eps", bufs=2, name=f"vv_{s}")[:, 0, 0:HD]
                for c in range(EC):
                    nc.tensor.matmul(
                        vv[:],
                        (x_sb[:, c, s * 128:(s + 1) * 128]),
                        (w_sb[:, c, 2 * HD:3 * HD]),
                        start=(c == 0), stop=(c == EC - 1))
                nc.vector.tensor_copy(
                    v_sb[:, s, 0:HL * (D + 1)].rearrange(
                        "p (h c) -> p h c", c=D + 1)[:, :, 0:D],
                    vv[:].rearrange("p (h d) -> p h d", h=HL))

        # ---- phase 2: attention ----
        with ExitStack() as ph2:
            esb_pool = ph2.enter_context(tc.tile_pool(name="esb", bufs=4))
            sm_pool = ph2.enter_context(tc.tile_pool(name="sm", bufs=4))

            for h in range(HL):
                m, p0 = h // 2, (h % 2) * 64
                avs = [psum_pool.tile([128, 512], F32, tag="av", bufs=4, name=f"av_{h}_{n}")
                       for n in range(NQ)]
                for kk in range(NK):
                    # two 2-bank energy tiles per kk so the next group's QK
                    # overlaps this group's exp (keeps the PE array gap-free;
                    # periodic array idles re-throttle the HAM clock gate)
                    e_sbs = []
                    for half in range(2):
                        e_ps = psum_pool.tile([128, 2, 512], F32, tag="eps", bufs=2,
                                             name=f"eps_{h}_{kk}_{half}")
                        for j in range(2):
                            n = half * 2 + j
                            nc.tensor.matmul(
                                e_ps[:, j, :],
                                (kT_sb[:, h, kk * 128:(kk + 1) * 128]),
                                (qT_sb[0:128, m, n * 512:(n + 1) * 512]),
                                start=True, stop=True)
                        e_sb = esb_pool.tile([128, 2, 512], BF16, tag="esb",
                                             name=f"esb_{h}_{kk}_{half}")
                        nc.scalar.activation(e_sb[:], e_ps[:], EXP, scale=SCALE)
                        e_sbs.append(e_sb)
                    for n in range(NQ):
                        nc.tensor.matmul(
                            avs[n][:],
                            (v_sb[:, kk, h * (D + 1): h * (D + 1) + 128]),
                            (e_sbs[n // 2][:, n % 2, :]),
                            start=(kk == 0), stop=(kk == NK - 1))
                # drain all four av banks first (the slow reciprocals would
                # otherwise sit ahead of the copies in the DVE queue and stall
                # the next head's AV accumulation on bank reuse). For the last
                # head, run per-n chains with the drain on the (idle) scalar
                # engine so phase 3 unblocks sooner.
                last = h == HL - 1
                avsts = []
                for n in range(NQ):
                    avst = sm_pool.tile([D + 1, 512], F32, tag="avst", bufs=8,
                                        name=f"avst_{h}_{n}")
                    if last:
                        nc.scalar.copy(avst[:], avs[n][0:D + 1, :])
                    else:
                        nc.vector.tensor_copy(avst[:], avs[n][0:D + 1, :])
                    avsts.append(avst)
                for n in range(NQ):
                    den = sm_pool.tile([1, 512], F32, tag="den", bufs=8,
                                       name=f"den_{h}_{n}")
                    nc.vector.tensor_copy(den[:], avsts[n][D:D + 1, :])
                    rec = sm_pool.tile([1, 512], F32, tag="rec", bufs=8,
                                       name=f"rec_{h}_{n}")
                    nc.vector.reciprocal_approx_fast(out=rec[:], in_=den[:])
                    rb = sm_pool.tile([D, 512], F32, tag="rb", bufs=8,
                                      name=f"rb_{h}_{n}")
                    nc.gpsimd.partition_broadcast(rb[:], rec[:])
                    nc.vector.tensor_mul(
                        attT_sb[p0:p0 + 64, m, n * 512:(n + 1) * 512],
                        avsts[n][0:D, :], rb[:])

        # keep the PE array busy through the last head's normalization tail
        # (an idle window >3.4us here re-throttles the clock for phase 3)
        fill_ps = psum_pool.tile([128, 512], F32, tag="av", bufs=4, name="fill_ps")
        for wi in range(40):
            nc.tensor.matmul(fill_ps[:], warm_sb[:, 0:128], warm_sb[:, 128:640],
                             start=(wi == 0), stop=(wi == 39))

        # ---- phase 3: output projection (natural layout) ----
        with ExitStack() as ph3:
            osb_pool = ph3.enter_context(tc.tile_pool(name="osb", bufs=3))
            for s in range(NK):
                o_sb = osb_pool.tile([128, EMB], F32, tag="osb", name=f"osb_{s}")
                for half in range(2):
                    pr = psum_pool.tile([128, 512], F32, tag="av", bufs=4, name=f"pr_{s}_{half}")[:, 0:HD]
                    for m in range(MC):
                        nc.tensor.matmul(
                            pr[:],
                            (attT_sb[:, m, s * 128:(s + 1) * 128]),
                            (wp_sb[:, m, half * HD:(half + 1) * HD]),
                            start=(m == 0), stop=(m == MC - 1))
                    nc.vector.tensor_copy(o_sb[:, half * HD:(half + 1) * HD], pr[:])
                nc.sync.dma_start(outr[:, s, :], o_sb[:])


_CACHE = {}


def _build():
    if "nc" not in _CACHE:
        nc = bacc.Bacc("TRN2", target_bir_lowering=False, debug=False,
                       num_devices=NCORES)
        with tile.TileContext(nc) as tc:
            _emit(tc)
        nc.compile()
        _CACHE["nc"] = nc
    return _CACHE["nc"]


def _in_maps(x, W_qkv, W_proj):
    in_maps = []
    for c in range(NCORES):
        b, g = divmod(c, 2)
        r0 = g * HD
        w_rows = np.concatenate([
            W_qkv[0 * EMB + r0: 0 * EMB + r0 + HD],
            W_qkv[1 * EMB + r0: 1 * EMB + r0 + HD],
            W_qkv[2 * EMB + r0: 2 * EMB + r0 + HD],
        ], axis=0)                                   # [1152, 768]
        bf = ml_dtypes.bfloat16
        in_maps.append({
            "xT": np.ascontiguousarray(x[b].T.astype(bf)),
            "wT": np.ascontiguousarray(w_rows.T.astype(bf)),
            "wpT": np.ascontiguousarray(W_proj[:, r0:r0 + HD].T.astype(bf)),
        })
    return in_maps


LAST_RESULTS = None


def kernel(x, W_qkv, W_proj, b_proj):
    global LAST_RESULTS
    x = np.ascontiguousarray(np.asarray(x, dtype=np.float32))
    W_qkv = np.asarray(W_qkv, dtype=np.float32)
    W_proj = np.asarray(W_proj, dtype=np.float32)
    b_proj = np.asarray(b_proj, dtype=np.float32)

    nc = _build()
    in_maps = _in_maps(x, W_qkv, W_proj)
    res = run_bass_kernel_spmd(nc, in_maps, core_ids=list(range(NCORES)))
    LAST_RESULTS = res

    out = np.empty((B, N, EMB), dtype=np.float32)
    for b in range(B):
        out[b] = res.results[2 * b]["outp"] + res.results[2 * b + 1]["outp"]
    out += b_proj
    return out



# revision 12
# speedup vs baseline: 1.6972x; 1.0057x over previous
"""MultiHeadAttention Trainium2 kernel.

Full inputs: x [4, 2048, 768] f32, W_qkv [2304, 768], W_proj [768, 768],
b_proj [768]. Output [4, 2048, 768] f32.

Sharding: 8 cores = 4 batches x 2 head-groups (6 heads each).
Per-core inputs (host-prepared, transposed on host):
  xT  [768, 2048]  = x[b].T
  wT  [768, 1152]  = concat(Wq_g, Wk_g, Wv_g).T   (g = head group rows)
  wpT [384, 768]   = W_proj[:, g-cols].T
Per-core output: outp [2048, 768] = partial projection output for batch b.
Host: out[b] = outp[2b] + outp[2b+1] + b_proj.

On-device (per core):
  phase 1: qT/kT [384, 2048] (head-dim on partitions) and v [2048, 384+ones]
           via f32r matmuls; x.T and W.T arrive pre-transposed from host.
  phase 2: per (head, k-chunk): energyT[k,q] = kT.T @ qT (K=64), one exp
           activation over 4 psum banks (scale=1/8 folded in, no max
           subtraction -- energies are O(+-10) for this distribution), then
           av[65, q] += v_aug.T @ e accumulated over k-chunks. Row 64 of av
           is the softmax denominator (ones column of v_aug).
           attT[hd, q] = av[0:64] * (1/l broadcast).
  phase 3: out[s, e] = attT.T @ wpT accumulated over hd-chunks -> DMA out.
"""

import ml_dtypes
import numpy as np

import concourse.bass as bass
import concourse.tile as tile
from concourse import bacc, mybir
from concourse.bass_utils import run_bass_kernel_spmd

EMB = 768
N = 2048
B = 4
D = 64
HL = 6            # heads per core
HD = HL * D       # 384 local head-dim columns
NCORES = 8
SCALE = D ** -0.5

F32 = mybir.dt.float32
BF16 = mybir.dt.bfloat16

EC = EMB // 128   # 6 emb chunks
MC = HD // 128    # 3 local head-dim chunks
NQ = N // 512     # 4 query chunks of 512
NK = N // 128     # 16 key/seq chunks of 128

EXP = mybir.ActivationFunctionType.Exp


def _emit(tc):
    from contextlib import ExitStack

    nc = tc.nc
    xT = nc.dram_tensor("xT", [EMB, N], BF16, kind="ExternalInput").ap()
    wT = nc.dram_tensor("wT", [EMB, 3 * HD], BF16, kind="ExternalInput").ap()
    wpT = nc.dram_tensor("wpT", [HD, EMB], BF16, kind="ExternalInput").ap()
    outp = nc.dram_tensor("outp", [N, EMB], F32, kind="ExternalOutput").ap()

    xTr = xT.rearrange("(c p) s -> p c s", p=128)
    wTr = wT.rearrange("(c p) s -> p c s", p=128)
    wpTr = wpT.rearrange("(m p) e -> p m e", p=128)
    outr = outp.rearrange("(s p) e -> p s e", p=128)

    with ExitStack() as persist:
        ppool = persist.enter_context(tc.tile_pool(name="persist", bufs=1))
        psum_pool0 = None  # placeholder, real pool created below
        # PE warmup: ~4us of junk matmuls, emitted first so they run during
        # the input-DMA wait and open the HAM clock-gate before real work
        warm_sb = ppool.tile([128, 640], BF16)
        nc.vector.memset(warm_sb[:], 1.0)
        wp_sb = ppool.tile([128, MC, EMB], BF16)
        nc.sync.dma_start(wp_sb[:], wpTr)
        qT_sb = ppool.tile([128, MC, N], BF16)
        kT_sb = ppool.tile([128, HL, N], BF16)
        nc.vector.memset(kT_sb[:], 0.0)
        v_sb = ppool.tile([128, NK, HL * (D + 1) + D], BF16)
        nc.vector.memset(v_sb[:], 1.0)
        attT_sb = ppool.tile([128, MC, N], BF16)

        psum_pool = persist.enter_context(
            tc.tile_pool(name="psum", bufs=1, space="PSUM"))
        warm_ps = psum_pool.tile([128, 512], F32, tag="av", bufs=4, name="warm_ps")
        for wi in range(10):
            nc.tensor.matmul(warm_ps[:], warm_sb[:, 0:128], warm_sb[:, 128:640],
                             start=(wi == 0), stop=(wi == 9))

        # ---- phase 1: qkv projection ----
        with ExitStack() as ph1:
            p1 = ph1.enter_context(tc.tile_pool(name="ph1", bufs=1))
            x_sb = p1.tile([128, EC, N], BF16)
            w_sb = p1.tile([128, EC, 3 * HD], BF16)
            for c in range(EC):
                nc.sync.dma_start(w_sb[:, c, :], wTr[:, c, :])
                nc.sync.dma_start(x_sb[:, c, :], xTr[:, c, :])

            for which in (0, 1):
                for m in range(MC):
                    lo = which * HD + m * 128
                    for n in range(NQ):
                        mm = psum_pool.tile([128, 512], F32, tag="av", bufs=4, name=f"mm_{which}_{m}_{n}")
                        for c in range(EC):
                            nc.tensor.matmul(
                                mm[:],
                                (w_sb[:, c, lo:lo + 128]),
                                (x_sb[:, c, n * 512:(n + 1) * 512]),
                                start=(c == 0), stop=(c == EC - 1))
                        ns = slice(n * 512, (n + 1) * 512)
                        if which == 0:
                            nc.vector.tensor_copy(qT_sb[:, m, ns], mm[:])
                        else:
                            nc.vector.tensor_copy(kT_sb[0:64, 2 * m, ns], mm[0:64, :])
                            nc.vector.tensor_copy(kT_sb[64:128, 2 * m + 1, ns], mm[64:128, :])

            for s in range(NK):
                vv = psum_pool.tile([128, 2, 512], F32, tag="eps", bufs=2, name=f"vv_{s}")[:, 0, 0:HD]
                for c in range(EC):
                    nc.tensor.matmul(
                        vv[:],
                        (x_sb[:, c, s * 128:(s + 1) * 128]),
                        (w_sb[:, c, 2 * HD:3 * HD]),
                        start=(c == 0), stop=(c == EC - 1))
                nc.vector.tensor_copy(
                    v_sb[:, s, 0:HL * (D + 1)].rearrange(
                        "p (h c) -> p h c", c=D + 1)[:, :, 0:D],
                    vv[:].rearrange("p (h d) -> p h d", h=HL))

        # ---- phase 2: attention ----
        with ExitStack() as ph2:
            esb_pool = ph2.enter_context(tc.tile_pool(name="esb", bufs=4))
            sm_pool = ph2.enter_context(tc.tile_pool(name="sm", bufs=4))

            for h in range(HL):
                m, p0 = h // 2, (h % 2) * 64
                avs = [psum_pool.tile([128, 512], F32, tag="av", bufs=4, name=f"av_{h}_{n}")
                       for n in range(NQ)]
                for kk in range(NK):
                    # two 2-bank energy tiles per kk so the next group's QK
                    # overlaps this group's exp (keeps the PE array gap-free;
                    # periodic array idles re-throttle the HAM clock gate)
                    e_sbs = []
                    for half in range(2):
                        e_ps = psum_pool.tile([128, 2, 512], F32, tag="eps", bufs=2,
                                             name=f"eps_{h}_{kk}_{half}")
                        for j in range(2):
                            n = half * 2 + j
                            nc.tensor.matmul(
                                e_ps[:, j, :],
                                (kT_sb[:, h, kk * 128:(kk + 1) * 128]),
                                (qT_sb[0:128, m, n * 512:(n + 1) * 512]),
                                start=True, stop=True)
                        e_sb = esb_pool.tile([128, 2, 512], BF16, tag="esb",
                                             name=f"esb_{h}_{kk}_{half}")
                        nc.scalar.activation(e_sb[:], e_ps[:], EXP, scale=SCALE)
                        e_sbs.append(e_sb)
                    for n in range(NQ):
                        nc.tensor.matmul(
                            avs[n][:],
                            (v_sb[:, kk, h * (D + 1): h * (D + 1) + 128]),
                            (e_sbs[n // 2][:, n % 2, :]),
                            start=(kk == 0), stop=(kk == NK - 1))
                # drain all four av banks first (the slow reciprocals would
                # otherwise sit ahead of the copies in the DVE queue and stall
                # the next head's AV accumulation on bank reuse). For the last
                # head, run per-n chains with the drain on the (idle) scalar
                # engine so phase 3 unblocks sooner.
                last = h == HL - 1
                avsts = []
                for n in range(NQ):
                    avst = sm_pool.tile([D + 1, 512], F32, tag="avst", bufs=8,
                                        name=f"avst_{h}_{n}")
                    if last:
                        nc.scalar.copy(avst[:], avs[n][0:D + 1, :])
                    else:
                        nc.vector.tensor_copy(avst[:], avs[n][0:D + 1, :])
                    avsts.append(avst)
                for n in range(NQ):
                    den = sm_pool.tile([1, 512], F32, tag="den", bufs=8,
                                       name=f"den_{h}_{n}")
                    nc.vector.tensor_copy(den[:], avsts[n][D:D + 1, :])
                    rec = sm_pool.tile([1, 512], F32, tag="rec", bufs=8,
                                       name=f"rec_{h}_{n}")
                    nc.vector.reciprocal_approx_fast(out=rec[:], in_=den[:])
                    rb = sm_pool.tile([D, 512], F32, tag="rb", bufs=8,
                                      name=f"rb_{h}_{n}")
                    nc.gpsimd.partition_broadcast(rb[:], rec[:])
                    nc.vector.tensor_mul(
                        attT_sb[p0:p0 + 64, m, n * 512:(n + 1) * 512],
                        avsts[n][0:D, :], rb[:])

        # keep the PE array busy through the last head's normalization tail
        # (an idle window >3.4us here re-throttles the clock for phase 3)
        fill_ps = psum_pool.tile([128, 512], F32, tag="av", bufs=4, name="fill_ps")
        for wi in range(40):
            nc.tensor.matmul(fill_ps[:], warm_sb[:, 0:128], warm_sb[:, 128:640],
                             start=(wi == 0), stop=(wi == 39))

        # ---- phase 3: output projection (natural layout) ----
        with ExitStack() as ph3:
            osb_pool = ph3.enter_context(tc.tile_pool(name="osb", bufs=3))
            for s in range(NK):
                o_sb = osb_pool.tile([128, EMB], F32, tag="osb", name=f"osb_{s}")
                for half in range(2):
                    pr = psum_pool.tile([128, 512], F32, tag="av", bufs=4, name=f"pr_{s}_{half}")[:, 0:HD]
                    for m in range(MC):
                        nc.tensor.matmul(
                            pr[:],
                            (attT_sb[:, m, s * 128:(s + 1) * 128]),
                            (wp_sb[:, m, half * HD:(half + 1) * HD]),
                            start=(m == 0), stop=(m == MC - 1))
                    nc.vector.tensor_copy(o_sb[:, half * HD:(half + 1) * HD], pr[:])
                nc.sync.dma_start(outr[:, s, :], o_sb[:])


_CACHE = {}


def _build():
    if "nc" not in _CACHE:
        nc = bacc.Bacc("TRN2", target_bir_lowering=False, debug=False,
                       num_devices=NCORES)
        with tile.TileContext(nc) as tc:
            _emit(tc)
        nc.compile()
        _CACHE["nc"] = nc
    return _CACHE["nc"]


def _in_maps(x, W_qkv, W_proj):
    in_maps = []
    for c in range(NCORES):
        b, g = divmod(c, 2)
        r0 = g * HD
        w_rows = np.concatenate([
            W_qkv[0 * EMB + r0: 0 * EMB + r0 + HD],
            W_qkv[1 * EMB + r0: 1 * EMB + r0 + HD],
            W_qkv[2 * EMB + r0: 2 * EMB + r0 + HD],
        ], axis=0)                                   # [1152, 768]
        bf = ml_dtypes.bfloat16
        in_maps.append({
            "xT": np.ascontiguousarray(x[b].T.astype(bf)),
            "wT": np.ascontiguousarray(w_rows.T.astype(bf)),
            "wpT": np.ascontiguousarray(W_proj[:, r0:r0 + HD].T.astype(bf)),
        })
    return in_maps


LAST_RESULTS = None


def kernel(x, W_qkv, W_proj, b_proj):
    global LAST_RESULTS
    x = np.ascontiguousarray(np.asarray(x, dtype=np.float32))
    W_qkv = np.asarray(W_qkv, dtype=np.float32)
    W_proj = np.asarray(W_proj, dtype=np.float32)
    b_proj = np.asarray(b_proj, dtype=np.float32)

    nc = _build()
    in_maps = _in_maps(x, W_qkv, W_proj)
    res = run_bass_kernel_spmd(nc, in_maps, core_ids=list(range(NCORES)))
    LAST_RESULTS = res

    out = np.empty((B, N, EMB), dtype=np.float32)
    for b in range(B):
        out[b] = res.results[2 * b]["outp"] + res.results[2 * b + 1]["outp"]
    out += b_proj
    return out



# revision 14
# speedup vs baseline: 1.7035x; 1.0037x over previous
"""MultiHeadAttention Trainium2 kernel.

Full inputs: x [4, 2048, 768] f32, W_qkv [2304, 768], W_proj [768, 768],
b_proj [768]. Output [4, 2048, 768] f32.

Sharding: 8 cores = 4 batches x 2 head-groups (6 heads each).
Per-core inputs (host-prepared, transposed on host):
  xT  [768, 2048]  = x[b].T
  wT  [768, 1152]  = concat(Wq_g, Wk_g, Wv_g).T   (g = head group rows)
  wpT [384, 768]   = W_proj[:, g-cols].T
Per-core output: outp [2048, 768] = partial projection output for batch b.
Host: out[b] = outp[2b] + outp[2b+1] + b_proj.

On-device (per core):
  phase 1: qT/kT [384, 2048] (head-dim on partitions) and v [2048, 384+ones]
           via f32r matmuls; x.T and W.T arrive pre-transposed from host.
  phase 2: per (head, k-chunk): energyT[k,q] = kT.T @ qT (K=64), one exp
           activation over 4 psum banks (scale=1/8 folded in, no max
           subtraction -- energies are O(+-10) for this distribution), then
           av[65, q] += v_aug.T @ e accumulated over k-chunks. Row 64 of av
           is the softmax denominator (ones column of v_aug).
           attT[hd, q] = av[0:64] * (1/l broadcast).
  phase 3: out[s, e] = attT.T @ wpT accumulated over hd-chunks -> DMA out.
"""

import ml_dtypes
import numpy as np

import concourse.bass as bass
import concourse.tile as tile
from concourse import bacc, mybir
from concourse.bass_utils import run_bass_kernel_spmd

EMB = 768
N = 2048
B = 4
D = 64
HL = 6            # heads per core
HD = HL * D       # 384 local head-dim columns
NCORES = 8
SCALE = D ** -0.5

F32 = mybir.dt.float32
BF16 = mybir.dt.bfloat16

EC = EMB // 128   # 6 emb chunks
MC = HD // 128    # 3 local head-dim chunks
NQ = N // 512     # 4 query chunks of 512
NK = N // 128     # 16 key/seq chunks of 128

EXP = mybir.ActivationFunctionType.Exp


def _emit(tc):
    from contextlib import ExitStack

    nc = tc.nc
    xT = nc.dram_tensor("xT", [EMB, N], BF16, kind="ExternalInput").ap()
    wT = nc.dram_tensor("wT", [EMB, 3 * HD], BF16, kind="ExternalInput").ap()
    wpT = nc.dram_tensor("wpT", [HD, EMB], BF16, kind="ExternalInput").ap()
    outp = nc.dram_tensor("outp", [N, EMB], F32, kind="ExternalOutput").ap()

    xTr = xT.rearrange("(c p) s -> p c s", p=128)
    wTr = wT.rearrange("(c p) s -> p c s", p=128)
    wpTr = wpT.rearrange("(m p) e -> p m e", p=128)
    outr = outp.rearrange("(s p) e -> p s e", p=128)

    with ExitStack() as persist:
        ppool = persist.enter_context(tc.tile_pool(name="persist", bufs=1))
        psum_pool0 = None  # placeholder, real pool created below
        # PE warmup: ~4us of junk matmuls, emitted first so they run during
        # the input-DMA wait and open the HAM clock-gate before real work
        warm_sb = ppool.tile([128, 640], BF16)
        nc.vector.memset(warm_sb[:], 1.0)
        wp_sb = ppool.tile([128, MC, EMB], BF16)
        nc.sync.dma_start(wp_sb[:], wpTr)
        qT_sb = ppool.tile([128, MC, N], BF16)
        kT_sb = ppool.tile([128, HL, N], BF16)
        nc.vector.memset(kT_sb[:], 0.0)
        v_sb = ppool.tile([128, NK, HL * (D + 1) + D], BF16)
        nc.vector.memset(v_sb[:], 1.0)
        attT_sb = ppool.tile([128, MC, N], BF16)

        psum_pool = persist.enter_context(
            tc.tile_pool(name="psum", bufs=1, space="PSUM"))
        warm_ps = psum_pool.tile([128, 512], F32, tag="av", bufs=4, name="warm_ps")
        for wi in range(10):
            nc.tensor.matmul(warm_ps[:], warm_sb[:, 0:128], warm_sb[:, 128:640],
                             start=(wi == 0), stop=(wi == 9))

        # ---- phase 1: qkv projection ----
        with ExitStack() as ph1:
            p1 = ph1.enter_context(tc.tile_pool(name="ph1", bufs=1))
            x_sb = p1.tile([128, EC, N], BF16)
            w_sb = p1.tile([128, EC, 3 * HD], BF16)
            for c in range(EC):
                nc.sync.dma_start(w_sb[:, c, :], wTr[:, c, :])
                nc.sync.dma_start(x_sb[:, c, :], xTr[:, c, :])

            for which in (0, 1):
                for m in range(MC):
                    lo = which * HD + m * 128
                    for n in range(NQ):
                        mm = psum_pool.tile([128, 512], F32, tag="av", bufs=4, name=f"mm_{which}_{m}_{n}")
                        for c in range(EC):
                            nc.tensor.matmul(
                                mm[:],
                                (w_sb[:, c, lo:lo + 128]),
                                (x_sb[:, c, n * 512:(n + 1) * 512]),
                                start=(c == 0), stop=(c == EC - 1))
                        ns = slice(n * 512, (n + 1) * 512)
                        if which == 0:
                            nc.vector.tensor_copy(qT_sb[:, m, ns], mm[:])
                        else:
                            nc.vector.tensor_copy(kT_sb[0:64, 2 * m, ns], mm[0:64, :])
                            nc.vector.tensor_copy(kT_sb[64:128, 2 * m + 1, ns], mm[64:128, :])

            for s in range(NK):
                vv = psum_pool.tile([128, 2, 512], F32, tag="eps", bufs=2, name=f"vv_{s}")[:, 0, 0:HD]
                for c in range(EC):
                    nc.tensor.matmul(
                        vv[:],
                        (x_sb[:, c, s * 128:(s + 1) * 128]),
                        (w_sb[:, c, 2 * HD:3 * HD]),
                        start=(c == 0), stop=(c == EC - 1))
                nc.vector.tensor_copy(
                    v_sb[:, s, 0:HL * (D + 1)].rearrange(
                        "p (h c) -> p h c", c=D + 1)[:, :, 0:D],
                    vv[:].rearrange("p (h d) -> p h d", h=HL))

        # ---- phase 2: attention ----
        with ExitStack() as ph2:
            esb_pool = ph2.enter_context(tc.tile_pool(name="esb", bufs=4))
            sm_pool = ph2.enter_context(tc.tile_pool(name="sm", bufs=4))
            osb_pool = ph2.enter_context(tc.tile_pool(name="osb", bufs=3))

            def ph3_s(s):
                # output projection for seq chunk s; psum from the eps tag
                # (the av tag is held by the last head's accumulators)
                o_sb = osb_pool.tile([128, EMB], F32, tag="osb",
                                     name=f"osb_{s}")
                for half in range(2):
                    pr = psum_pool.tile([128, 2, 512], F32, tag="eps", bufs=2,
                                        name=f"pr_{s}_{half}")[:, 0, 0:HD]
                    for mi in range(MC):
                        nc.tensor.matmul(
                            pr[:],
                            (attT_sb[:, mi, s * 128:(s + 1) * 128]),
                            (wp_sb[:, mi, half * HD:(half + 1) * HD]),
                            start=(mi == 0), stop=(mi == MC - 1))
                    nc.vector.tensor_copy(o_sb[:, half * HD:(half + 1) * HD],
                                          pr[:])
                nc.sync.dma_start(outr[:, s, :], o_sb[:])

            for h in range(HL):
                m, p0 = h // 2, (h % 2) * 64
                avs = [psum_pool.tile([128, 512], F32, tag="av", bufs=4, name=f"av_{h}_{n}")
                       for n in range(NQ)]
                for kk in range(NK):
                    # two 2-bank energy tiles per kk so the next group's QK
                    # overlaps this group's exp (keeps the PE array gap-free;
                    # periodic array idles re-throttle the HAM clock gate)
                    e_sbs = []
                    for half in range(2):
                        e_ps = psum_pool.tile([128, 2, 512], F32, tag="eps", bufs=2,
                                             name=f"eps_{h}_{kk}_{half}")
                        for j in range(2):
                            n = half * 2 + j
                            nc.tensor.matmul(
                                e_ps[:, j, :],
                                (kT_sb[:, h, kk * 128:(kk + 1) * 128]),
                                (qT_sb[0:128, m, n * 512:(n + 1) * 512]),
                                start=True, stop=True)
                        e_sb = esb_pool.tile([128, 2, 512], BF16, tag="esb",
                                             name=f"esb_{h}_{kk}_{half}")
                        nc.scalar.activation(e_sb[:], e_ps[:], EXP, scale=SCALE)
                        e_sbs.append(e_sb)
                    for n in range(NQ):
                        nc.tensor.matmul(
                            avs[n][:],
                            (v_sb[:, kk, h * (D + 1): h * (D + 1) + 128]),
                            (e_sbs[n // 2][:, n % 2, :]),
                            start=(kk == 0), stop=(kk == NK - 1))
                # drain all four av banks first (the slow reciprocals would
                # otherwise sit ahead of the copies in the DVE queue and stall
                # the next head's AV accumulation on bank reuse). For the last
                # head, run per-n chains with the drain on the (idle) scalar
                # engine so phase 3 unblocks sooner.
                last = h == HL - 1
                if last:
                    # bridge the PE idle between the last AV chain and the
                    # first interleaved phase-3 chunk (idle >3.4us would
                    # re-throttle the clock)
                    fill_ps = psum_pool.tile([128, 2, 512], F32, tag="eps",
                                             bufs=2, name="fill_ps")
                    for wi in range(12):
                        nc.tensor.matmul(fill_ps[:, 0, :], warm_sb[:, 0:128],
                                         warm_sb[:, 128:640],
                                         start=(wi == 0), stop=(wi == 11))
                avsts = []
                for n in range(NQ):
                    avst = sm_pool.tile([D + 1, 512], F32, tag="avst", bufs=8,
                                        name=f"avst_{h}_{n}")
                    if last:
                        nc.scalar.copy(avst[:], avs[n][0:D + 1, :])
                    else:
                        nc.vector.tensor_copy(avst[:], avs[n][0:D + 1, :])
                    avsts.append(avst)
                for n in range(NQ):
                    den = sm_pool.tile([1, 512], F32, tag="den", bufs=8,
                                       name=f"den_{h}_{n}")
                    nc.vector.tensor_copy(den[:], avsts[n][D:D + 1, :])
                    rec = sm_pool.tile([1, 512], F32, tag="rec", bufs=8,
                                       name=f"rec_{h}_{n}")
                    nc.vector.reciprocal_approx_fast(out=rec[:], in_=den[:])
                    rb = sm_pool.tile([D, 512], F32, tag="rb", bufs=8,
                                      name=f"rb_{h}_{n}")
                    nc.gpsimd.partition_broadcast(rb[:], rec[:])
                    nc.vector.tensor_mul(
                        attT_sb[p0:p0 + 64, m, n * 512:(n + 1) * 512],
                        avsts[n][0:D, :], rb[:])
                    if last:
                        for s_ in range(4 * n, 4 * n + 4):
                            ph3_s(s_)




_CACHE = {}


def _build():
    if "nc" not in _CACHE:
        nc = bacc.Bacc("TRN2", target_bir_lowering=False, debug=False,
                       num_devices=NCORES)
        with tile.TileContext(nc) as tc:
            _emit(tc)
        nc.compile()
        _CACHE["nc"] = nc
    return _CACHE["nc"]


def _in_maps(x, W_qkv, W_proj):
    in_maps = []
    for c in range(NCORES):
        b, g = divmod(c, 2)
        r0 = g * HD
        w_rows = np.concatenate([
            W_qkv[0 * EMB + r0: 0 * EMB + r0 + HD],
            W_qkv[1 * EMB + r0: 1 * EMB + r0 + HD],
            W_qkv[2 * EMB + r0: 2 * EMB + r0 + HD],
        ], axis=0)                                   # [1152, 768]
        bf = ml_dtypes.bfloat16
        in_maps.append({
            "xT": np.ascontiguousarray(x[b].T.astype(bf)),
            "wT": np.ascontiguousarray(w_rows.T.astype(bf)),
            "wpT": np.ascontiguousarray(W_proj[:, r0:r0 + HD].T.astype(bf)),
        })
    return in_maps


LAST_RESULTS = None


def kernel(x, W_qkv, W_proj, b_proj):
    global LAST_RESULTS
    x = np.ascontiguousarray(np.asarray(x, dtype=np.float32))
    W_qkv = np.asarray(W_qkv, dtype=np.float32)
    W_proj = np.asarray(W_proj, dtype=np.float32)
    b_proj = np.asarray(b_proj, dtype=np.float32)

    nc = _build()
    in_maps = _in_maps(x, W_qkv, W_proj)
    res = run_bass_kernel_spmd(nc, in_maps, core_ids=list(range(NCORES)))
    LAST_RESULTS = res

    out = np.empty((B, N, EMB), dtype=np.float32)
    for b in range(B):
        out[b] = res.results[2 * b]["outp"] + res.results[2 * b + 1]["outp"]
    out += b_proj
    return out

